# revision 22
# baseline (speedup 1.0000x reference)
"""Causal self-attention (B=2, T=2048, E=1024, H=16) on 8 trn2 NeuronCores.

Sharding: core = b*4 + g  (b = batch index, g = head-group of 4 heads).
Each core computes its 4 heads' attention for its batch plus a partial
output projection; the host sums the 4 partials per batch.

v2 structure (two phases, PSUM decoupled):
  Phase 1 (projections): q/k plain projections accumulate in PSUM; the
  rotate_half for RoPE is a single 128x128 block-diag permutation
  matmul on the bf16 copy of the plain result (replaces a second full
  E-contraction projection).  ACT does the psum->sbuf copies (it is
  otherwise idle in this phase); DVE does the RoPE combines.
  Phase 2 (attention): scores computed transposed (S^T = K Q^T, k on
  partitions) into a dedicated double-buffered PSUM pool; exp on ACT;
  V carries an appended ones column so row 64 of the attention psum is
  the softmax denominator.  Output projection per 128-q-row chunk
  produces full-E rows -> one big contiguous DMA per chunk; its psum
  tile is borrowed from the score pool.  Diagonal chunks trim the
  causally-dead columns out of the score matmul, exp, and AV matmul.
"""

import numpy as np
import ml_dtypes

BF16 = ml_dtypes.bfloat16

B, T, E = 2, 2048, 1024
H, HD = 16, 64
G = 4             # head groups (cores per batch)
HL = H // G       # heads per core
DL = HL * HD      # local qkv dim = 256
TC = 512          # T chunk (matmul moving free dim)
NJ = T // TC      # 4 q-windows
KC = 128          # k-chunk (contraction tile for attention)
NC_ = T // KC     # 16 k-chunks
SCALE = 1.0 / float(np.sqrt(HD))

_CACHE = {}


def _build_bass():
    import concourse.mybir as mybir
    import concourse.tile as tile
    from concourse import bacc

    f32 = mybir.dt.float32
    bf16 = mybir.dt.bfloat16
    EXP = mybir.ActivationFunctionType.Exp

    nc = bacc.Bacc("TRN2", target_bir_lowering=False, debug=False)
    xt_d = nc.dram_tensor("xt", [E, T], bf16, kind="ExternalInput").ap()
    w_d = nc.dram_tensor("w", [E, 3 * DL], bf16, kind="ExternalInput").ap()
    wo_d = nc.dram_tensor("wo", [DL, E], bf16, kind="ExternalInput").ap()
    cos_d = nc.dram_tensor("cosf", [128, T], bf16, kind="ExternalInput").ap()
    sin_d = nc.dram_tensor("sinf", [128, T], bf16, kind="ExternalInput").ap()
    rm_d = nc.dram_tensor("rmat", [128, 128], bf16, kind="ExternalInput").ap()
    y_d = nc.dram_tensor("y", [T, E], bf16, kind="ExternalOutput").ap()

    NKK = E // KC  # 8 contraction chunks for the projections

    with tile.TileContext(nc) as tc:
        with (
            tc.tile_pool(name="consts", bufs=1) as consts,
            tc.tile_pool(name="stp", bufs=2, space="PSUM") as stp,     # proj/scores/yp
            tc.tile_pool(name="avp", bufs=1, space="PSUM") as avp,     # v proj / AV
            tc.tile_pool(name="sbp", bufs=3) as sbp,                   # plain bf16
            tc.tile_pool(name="tmp_sb", bufs=3) as tmp_sb,             # rope tmps
            tc.tile_pool(name="est_sb", bufs=6) as est_sb,
            tc.tile_pool(name="attn_sb", bufs=2) as attn_sb,
            tc.tile_pool(name="ysb_p", bufs=2) as ysb_p,
            tc.tile_pool(name="small_sb", bufs=3) as small_sb,
        ):
            # ---- constant tiles (DMAs ordered for earliest first use) ----
            w = [consts.tile([KC, 3 * DL], bf16, tag=f"w{i}", name=f"w{i}")
                 for i in range(NKK)]
            xt = [consts.tile([KC, T], bf16, tag=f"xt{i}", name=f"xt{i}")
                  for i in range(NKK)]
            cosf = consts.tile([128, T], bf16, tag="cosf")
            sinf = consts.tile([128, T], bf16, tag="sinf")
            rmat = consts.tile([128, 128], bf16, tag="rmat")

            # first projection (q tau0, window 0) needs w cols 0:128 and
            # xt window 0; stream those first, then just-in-time order.
            for i in range(NKK):
                nc.sync.dma_start(out=w[i][:, 0:128], in_=w_d[i * KC:(i + 1) * KC, 0:128])
                nc.sync.dma_start(out=xt[i][:, 0:TC], in_=xt_d[i * KC:(i + 1) * KC, 0:TC])
            nc.sync.dma_start(out=rmat, in_=rm_d)
            nc.sync.dma_start(out=cosf[:, 0:TC], in_=cos_d[:, 0:TC])
            nc.sync.dma_start(out=sinf[:, 0:TC], in_=sin_d[:, 0:TC])
            for blk in range(1, 6):
                bs = slice(blk * 128, (blk + 1) * 128)
                for i in range(NKK):
                    nc.sync.dma_start(out=w[i][:, bs], in_=w_d[i * KC:(i + 1) * KC, bs])
            for jj in range(1, NJ):
                js = slice(jj * TC, (jj + 1) * TC)
                for i in range(NKK):
                    nc.sync.dma_start(out=xt[i][:, js], in_=xt_d[i * KC:(i + 1) * KC, js])
                nc.sync.dma_start(out=cosf[:, js], in_=cos_d[:, js])
                nc.sync.dma_start(out=sinf[:, js], in_=sin_d[:, js])
            wo = []
            for tau in range(2):
                t = consts.tile([128, E], bf16, tag=f"wo{tau}", name=f"wo{tau}")
                nc.sync.dma_start(out=t, in_=wo_d[tau * 128:(tau + 1) * 128, :])
                wo.append(t)

            # triangular band mask: band[p, f] = 1 if f >= p else 0
            band = consts.tile([128, KC], bf16, tag="band")
            nc.gpsimd.memset(band, 1.0)
            nc.gpsimd.affine_select(
                out=band, in_=band, compare_op=mybir.AluOpType.is_ge, fill=0.0,
                base=0, pattern=[[1, KC]], channel_multiplier=-1,
            )

            # resident projection outputs (natural head-contiguous layout)
            qn = [[consts.tile([128, TC], bf16, tag=f"qn{tau}_{j}",
                               name=f"qn{tau}_{j}") for j in range(NJ)]
                  for tau in range(2)]
            kn = [[consts.tile([128, TC], bf16, tag=f"kn{tau}_{j}",
                               name=f"kn{tau}_{j}") for j in range(NJ)]
                  for tau in range(2)]
            vsb = [consts.tile([128, HL * 65], bf16, tag=f"v{c}", name=f"v{c}")
                   for c in range(NC_)]
            for c in range(NC_):
                vv = vsb[c].rearrange("p (h d) -> p h d", h=HL)
                nc.gpsimd.memset(vv[:, :, 64:65], 1.0)

            # ---------------- phase 1: projections ----------------
            # The rotate-half permutation matmul of part p is emitted after
            # part p+1's plain chain: it depends on an ACT copy of part p's
            # psum, and the in-order PE queue would stall on it otherwise.
            rot_pend = []

            def flush_rot(keep=0):
                while len(rot_pend) > keep:
                    pr, sb, ta, tb, dstt, js = rot_pend.pop(0)
                    nc.tensor.matmul(pr, lhsT=rmat, rhs=sb,
                                     start=True, stop=True)
                    nc.vector.tensor_mul(ta, sb, cosf[:, js])
                    nc.vector.tensor_mul(tb, pr, sinf[:, js])
                    nc.vector.tensor_add(dstt, ta, tb)

            def emit_proj_qk(jj, base, tau, dst):
                """Plain projection -> bf16 copy -> permutation matmul for
                rotate_half -> RoPE combine into dst[tau][jj]."""
                js = slice(jj * TC, (jj + 1) * TC)
                cc = base + 128 * tau
                p2 = stp.tile([128, 2 * TC], f32, tag="st",
                              name=f"pp{base}_{tau}_{jj}")
                ps = p2[:, 0:TC]
                pr = p2[:, TC:2 * TC]
                for kk in range(NKK):
                    nc.tensor.matmul(
                        ps, lhsT=w[kk][:, cc:cc + 128], rhs=xt[kk][:, js],
                        start=(kk == 0), stop=(kk == NKK - 1))
                sb = sbp.tile([128, TC], bf16, tag="sb", name=f"sb{base}_{tau}_{jj}")
                nc.scalar.copy(sb, ps)
                ta = tmp_sb.tile([128, TC], bf16, tag="ropeA",
                                 name=f"ra{base}_{tau}_{jj}")
                tb = tmp_sb.tile([128, TC], bf16, tag="ropeB",
                                 name=f"rb{base}_{tau}_{jj}")
                rot_pend.append((pr, sb, ta, tb, dst[tau][jj], js))
                flush_rot(keep=1)

            def emit_proj_v(jj):
                js0 = jj * (TC // KC)
                ps = stp.tile([128, 2 * TC], f32, tag="st", name=f"pv{jj}")
                for tt in range(TC // KC):
                    c = js0 + tt
                    for kk in range(NKK):
                        nc.tensor.matmul(
                            ps[:, tt * DL:(tt + 1) * DL],
                            lhsT=xt[kk][:, c * KC:(c + 1) * KC],
                            rhs=w[kk][:, 2 * DL:3 * DL],
                            start=(kk == 0), stop=(kk == NKK - 1))
                for tt in range(TC // KC):
                    c = js0 + tt
                    vv = vsb[c].rearrange("p (h d) -> p h d", h=HL)
                    nc.vector.tensor_copy(
                        vv[:, :, 0:64],
                        ps[:, tt * DL:(tt + 1) * DL].rearrange(
                            "p (h d) -> p h d", h=HL))

            # phase 1 proper: q/k projections (+ v of window 0); the other
            # v projections ride inside the attention chunk loops where the
            # PE has slack under the ACT-bound exp cadence
            for jj in range(NJ):
                for tau in range(2):
                    emit_proj_qk(jj, 0, tau, qn)
                for tau in range(2):
                    emit_proj_qk(jj, DL, tau, kn)
                if jj == 0:
                    emit_proj_v(0)
            flush_rot()

            # ---------------- phase 2: attention ----------------
            def emit_y(jj, tt):
                """Output projection for q rows [jj*TC + tt*128, +128): full
                E columns, one contiguous DMA."""
                at = ats[jj]
                yp = stp.tile([128, 2 * TC], f32, tag="st", name=f"yp{jj}_{tt}")
                for n in range(2):
                    for tau in range(2):
                        nc.tensor.matmul(
                            yp[:, n * TC:(n + 1) * TC],
                            lhsT=at[tau][:, tt * KC:(tt + 1) * KC],
                            rhs=wo[tau][:, n * TC:(n + 1) * TC],
                            start=(tau == 0), stop=(tau == 1))
                ys = ysb_p.tile([128, 2 * TC], bf16, tag="y", name=f"ys{jj}_{tt}")
                nc.vector.tensor_copy(ys, yp)
                nc.sync.dma_start(
                    out=y_d[jj * TC + tt * KC:jj * TC + (tt + 1) * KC, :],
                    in_=ys)

            ats = []
            for j in range(NJ):
                nch = 4 * (j + 1)          # causal k-chunks for this window
                # previous window's 4 output-projection chunks go mid-window
                # (late enough that at[j-1] is certainly done: the in-order
                # PE queue would otherwise stall behind a premature y matmul)
                y_at = {}
                if j > 0:
                    for tt in range(4):
                        y_at.setdefault(
                            min(nch - 1, 5 + tt * max(1, (nch - 5) // 4)),
                            []).append(tt)

                at = [attn_sb.tile([128, TC], bf16, tag=f"attn{tau}",
                                   name=f"attn{tau}_{j}") for tau in range(2)]
                ats.append(at)
                av4 = avp.tile([128, 4 * TC], f32, tag="av", name=f"av_{j}")

                def emit_av(c):
                    coff = KC * (c - 4 * j) if c > 4 * j else 0
                    for h in range(HL):
                        nc.tensor.matmul(
                            av4[0:65, h * TC + coff:(h + 1) * TC],
                            lhsT=vsb[c][:, 65 * h:65 * h + 65],
                            rhs=av_est[c][h // 2][:, (h % 2) * TC + coff:
                                                  (h % 2 + 1) * TC],
                            start=(c == 0), stop=(c == nch - 1))

                av_est = {}
                for c in range(nch):
                    d = c - 4 * j          # 0..3 on the diagonal
                    coff = KC * d if d > 0 else 0
                    sts = [stp.tile([128, 2 * TC], f32, tag="st",
                                    name=f"st{j}_{c}_{i}") for i in range(2)]
                    # one K=64 matmul per head; the two heads of a tile sit
                    # on disjoint row-groups of the PE array
                    for ll in range(2):
                        for tau in range(2):
                            h = 2 * tau + ll
                            stt = sts[h // 2]
                            w0_ = (h % 2) * TC
                            nc.tensor.matmul(
                                stt[:, w0_ + coff:w0_ + TC],
                                lhsT=kn[tau][c // 4][
                                    64 * ll:64 * ll + 64,
                                    (c % 4) * KC:(c % 4 + 1) * KC],
                                rhs=qn[tau][j][64 * ll:64 * ll + 64, coff:],
                                start=True, stop=True,
                                tile_position=(64 * ll, 0))
                    ests = []
                    for i in range(2):
                        est = est_sb.tile([128, 2 * TC], bf16, tag="est",
                                          name=f"est{j}_{c}_{i}")
                        if coff:
                            nc.scalar.activation(
                                est.rearrange("p (w c) -> p w c", w=2)[
                                    :, :, coff:],
                                sts[i].rearrange("p (w c) -> p w c", w=2)[
                                    :, :, coff:],
                                EXP, scale=SCALE)
                        else:
                            nc.scalar.activation(est, sts[i], EXP, scale=SCALE)
                        ests.append(est)
                    if d >= 0:
                        for h in range(HL):
                            bs = slice((h % 2) * TC + KC * d,
                                       (h % 2) * TC + KC * (d + 1))
                            nc.vector.tensor_mul(
                                ests[h // 2][:, bs], ests[h // 2][:, bs], band)
                    av_est[c] = ests
                    # AV lagged two chunks behind the scores: keeps the PE
                    # queue fed with score matmuls while the window-boundary
                    # normalize frees the av4 accumulator (early chunks have
                    # no band-mul, so est readiness is ACT-paced only)
                    if c >= 2:
                        emit_av(c - 2)
                    if c == 2 and j + 1 < NJ:
                        emit_proj_v(j + 1)
                    for tt in y_at.get(c, ()):
                        emit_y(j - 1, tt)
                emit_av(nch - 2)
                emit_av(nch - 1)

                # softmax normalize: copy the raw sums out of psum (frees
                # av4 for the next window), then scale by the reciprocal of
                # the denominator row.  Odd heads go through a base-0 temp
                # tile: SBUF*SBUF tensor ops need equal input base
                # partitions (output partition offset is fine).
                last = j == NJ - 1

                def stage_raw():
                    tmps = {}
                    for h in range(HL):
                        hw = slice(h * TC, (h + 1) * TC)
                        if h % 2 == 0:
                            nc.vector.tensor_copy(at[h // 2][0:64, :],
                                                  av4[0:64, hw])
                        else:
                            tmp = small_sb.tile([64, TC], bf16,
                                                tag=f"atmp{h}",
                                                name=f"atmp{j}_{h}")
                            tmps[h] = tmp
                            nc.vector.tensor_copy(tmp, av4[0:64, hw])
                    return tmps

                def stage_recip():
                    rbs = []
                    for h in range(HL):
                        hw = slice(h * TC, (h + 1) * TC)
                        avd = small_sb.tile([1, TC], f32, tag="denom",
                                            name=f"avd{j}_{h}")
                        nc.vector.tensor_copy(avd, av4[64:65, hw])
                        rc = small_sb.tile([1, TC], f32, tag="recip",
                                           name=f"rc{j}_{h}")
                        nc.vector.reciprocal_approx_fast(out=rc, in_=avd)
                        rb = small_sb.tile([64, TC], f32, tag=f"rbcast{h}",
                                           name=f"rb{j}_{h}")
                        nc.gpsimd.partition_broadcast(rb, rc)
                        rbs.append(rb)
                    return rbs

                if not last:
                    # free av4 as early as possible; at-readiness has slack
                    # (y parts sit mid-next-window)
                    tmps = stage_raw()
                    rbs = stage_recip()
                    for h in range(HL):
                        dst = at[h // 2][64 * (h % 2):64 * (h % 2) + 64, :]
                        src = dst if h % 2 == 0 else tmps[h]
                        nc.vector.tensor_mul(dst, src, rbs[h])
                else:
                    # tail: at-readiness is the critical path; everything is
                    # column-split so the first output projection fires as
                    # early as possible, straight out of psum
                    for tt in range(4):
                        ks = slice(tt * KC, (tt + 1) * KC)
                        rbs = []
                        for h in range(HL):
                            co = h * TC + tt * KC
                            avd = small_sb.tile([1, KC], f32, tag="denom",
                                                name=f"avd{j}_{h}_{tt}")
                            nc.vector.tensor_copy(avd, av4[64:65, co:co + KC])
                            rc = small_sb.tile([1, KC], f32, tag="recip",
                                               name=f"rc{j}_{h}_{tt}")
                            nc.vector.reciprocal_approx_fast(out=rc, in_=avd)
                            rb = small_sb.tile([64, KC], f32,
                                               tag=f"rbcast{h}",
                                               name=f"rb{j}_{h}_{tt}")
                            nc.gpsimd.partition_broadcast(rb, rc)
                            rbs.append(rb)
                        for h in range(HL):
                            co = h * TC + tt * KC
                            nc.vector.tensor_mul(
                                at[h // 2][64 * (h % 2):64 * (h % 2) + 64,
                                           ks],
                                av4[0:64, co:co + KC], rbs[h])
                        emit_y(j, tt)

    nc.compile()
    return nc


def _host_inputs(x, cos, sin, w_qkv, w_out):
    """Shard + lay out the full inputs for the 8 cores."""
    # natural-layout tables: row 64*l + d = cos/sin[t, d]
    cosf = np.ascontiguousarray(np.tile(cos.T, (2, 1))).astype(BF16)
    sinf = np.ascontiguousarray(np.tile(sin.T, (2, 1))).astype(BF16)

    xts = [np.ascontiguousarray(x[b].T).astype(BF16) for b in range(B)]

    # lhsT for the rotate_half permutation matmul: rot = rmat.T @ q with
    # rot[d] = -q[d+32] (d<32), q[d-32] (d>=32) per 64-row head block
    r64 = np.zeros((64, 64), dtype=np.float32)
    r64[np.arange(32) + 32, np.arange(32)] = -1.0
    r64[np.arange(32), np.arange(32) + 32] = 1.0
    rmat = np.zeros((128, 128), dtype=np.float32)
    rmat[0:64, 0:64] = r64
    rmat[64:128, 64:128] = r64
    rmat = rmat.astype(BF16)

    in_maps = []
    for core in range(8):
        b, g = divmod(core, G)
        qblk = w_qkv[:, G * g * HD:(G * g + HL) * HD]
        kblk = w_qkv[:, E + G * g * HD:E + (G * g + HL) * HD]
        vblk = w_qkv[:, 2 * E + DL * g:2 * E + DL * (g + 1)]
        wl = np.concatenate([qblk, kblk, vblk], axis=1).astype(BF16)  # (E, 768)
        wol = np.ascontiguousarray(w_out[DL * g:DL * (g + 1), :]).astype(BF16)
        in_maps.append({
            "xt": xts[b], "w": wl, "wo": wol, "cosf": cosf, "sinf": sinf,
            "rmat": rmat,
        })
    return in_maps


def kernel(x, cos, sin, w_qkv, w_out):
    from concourse import bass_utils

    if "nc" not in _CACHE:
        _CACHE["nc"] = _build_bass()
    nc = _CACHE["nc"]

    in_maps = _host_inputs(
        np.asarray(x, dtype=np.float32), np.asarray(cos, dtype=np.float32),
        np.asarray(sin, dtype=np.float32), np.asarray(w_qkv, dtype=np.float32),
        np.asarray(w_out, dtype=np.float32))

    res = bass_utils.run_bass_kernel_spmd(nc, in_maps, core_ids=list(range(8)))

    y = np.zeros((B, T, E), dtype=np.float32)
    for core in range(8):
        b = core // G
        y[b] += res.results[core]["y"].astype(np.float32)
    return y


# revision 26
# speedup vs baseline: 1.0368x; 1.0368x over previous
"""Causal self-attention (B=2, T=2048, E=1024, H=16) on 8 trn2 NeuronCores.

Sharding: core = b*4 + g  (b = batch index, g = head-group of 4 heads).
Each core computes its 4 heads' attention for its batch plus a partial
output projection; the host sums the 4 partials per batch.

v2 structure (two phases, PSUM decoupled):
  Phase 1 (projections): q/k plain projections accumulate in PSUM; the
  rotate_half for RoPE is a single 128x128 block-diag permutation
  matmul on the bf16 copy of the plain result (replaces a second full
  E-contraction projection).  ACT does the psum->sbuf copies (it is
  otherwise idle in this phase); DVE does the RoPE combines.
  Phase 2 (attention): scores computed transposed (S^T = K Q^T, k on
  partitions) into a dedicated double-buffered PSUM pool; exp on ACT;
  V carries an appended ones column so row 64 of the attention psum is
  the softmax denominator.  Output projection per 128-q-row chunk
  produces full-E rows -> one big contiguous DMA per chunk; its psum
  tile is borrowed from the score pool.  Diagonal chunks trim the
  causally-dead columns out of the score matmul, exp, and AV matmul.
"""

import numpy as np
import ml_dtypes

BF16 = ml_dtypes.bfloat16

B, T, E = 2, 2048, 1024
H, HD = 16, 64
G = 4             # head groups (cores per batch)
HL = H // G       # heads per core
DL = HL * HD      # local qkv dim = 256
TC = 512          # T chunk (matmul moving free dim)
NJ = T // TC      # 4 q-windows
KC = 128          # k-chunk (contraction tile for attention)
NC_ = T // KC     # 16 k-chunks
SCALE = 1.0 / float(np.sqrt(HD))

_CACHE = {}


def _build_bass():
    import concourse.mybir as mybir
    import concourse.tile as tile
    from concourse import bacc

    f32 = mybir.dt.float32
    bf16 = mybir.dt.bfloat16
    EXP = mybir.ActivationFunctionType.Exp

    nc = bacc.Bacc("TRN2", target_bir_lowering=False, debug=False)
    xt_d = nc.dram_tensor("xt", [E, T], bf16, kind="ExternalInput").ap()
    w_d = nc.dram_tensor("w", [E, 3 * DL], bf16, kind="ExternalInput").ap()
    wo_d = nc.dram_tensor("wo", [DL, E], bf16, kind="ExternalInput").ap()
    cos_d = nc.dram_tensor("cosf", [128, T], bf16, kind="ExternalInput").ap()
    sin_d = nc.dram_tensor("sinf", [128, T], bf16, kind="ExternalInput").ap()
    rm_d = nc.dram_tensor("rmat", [128, 128], bf16, kind="ExternalInput").ap()
    y_d = nc.dram_tensor("y", [T, E], bf16, kind="ExternalOutput").ap()

    NKK = E // KC  # 8 contraction chunks for the projections

    with tile.TileContext(nc) as tc:
        with (
            tc.tile_pool(name="consts", bufs=1) as consts,
            tc.tile_pool(name="stp", bufs=2, space="PSUM") as stp,     # proj/scores/yp
            tc.tile_pool(name="avp", bufs=1, space="PSUM") as avp,     # v proj / AV
            tc.tile_pool(name="sbp", bufs=3) as sbp,                   # plain bf16
            tc.tile_pool(name="tmp_sb", bufs=3) as tmp_sb,             # rope tmps
            tc.tile_pool(name="est_sb", bufs=6) as est_sb,
            tc.tile_pool(name="attn_sb", bufs=2) as attn_sb,
            tc.tile_pool(name="ysb_p", bufs=2) as ysb_p,
            tc.tile_pool(name="small_sb", bufs=3) as small_sb,
        ):
            # ---- constant tiles (DMAs ordered for earliest first use) ----
            w = [consts.tile([KC, 3 * DL], bf16, tag=f"w{i}", name=f"w{i}")
                 for i in range(NKK)]
            xt = [consts.tile([KC, T], bf16, tag=f"xt{i}", name=f"xt{i}")
                  for i in range(NKK)]
            cosf = consts.tile([128, T], bf16, tag="cosf")
            sinf = consts.tile([128, T], bf16, tag="sinf")
            rmat = consts.tile([128, 128], bf16, tag="rmat")

            # the v projection runs first (it only needs the v column
            # groups 4/5), absorbing the DMA latency of the q/k groups
            for i in range(NKK):
                nc.sync.dma_start(out=w[i][:, 512:768],
                                  in_=w_d[i * KC:(i + 1) * KC, 512:768])
                nc.sync.dma_start(out=xt[i][:, 0:TC], in_=xt_d[i * KC:(i + 1) * KC, 0:TC])
            for i in range(NKK):
                nc.sync.dma_start(out=w[i][:, 0:128], in_=w_d[i * KC:(i + 1) * KC, 0:128])
            nc.sync.dma_start(out=rmat, in_=rm_d)
            nc.sync.dma_start(out=cosf[:, 0:TC], in_=cos_d[:, 0:TC])
            nc.sync.dma_start(out=sinf[:, 0:TC], in_=sin_d[:, 0:TC])
            for blk in range(1, 4):
                bs = slice(blk * 128, (blk + 1) * 128)
                for i in range(NKK):
                    nc.sync.dma_start(out=w[i][:, bs], in_=w_d[i * KC:(i + 1) * KC, bs])
            for jj in range(1, NJ):
                js = slice(jj * TC, (jj + 1) * TC)
                for i in range(NKK):
                    nc.sync.dma_start(out=xt[i][:, js], in_=xt_d[i * KC:(i + 1) * KC, js])
                nc.sync.dma_start(out=cosf[:, js], in_=cos_d[:, js])
                nc.sync.dma_start(out=sinf[:, js], in_=sin_d[:, js])
            wo = []
            for tau in range(2):
                t = consts.tile([128, E], bf16, tag=f"wo{tau}", name=f"wo{tau}")
                nc.sync.dma_start(out=t, in_=wo_d[tau * 128:(tau + 1) * 128, :])
                wo.append(t)

            # triangular band mask: band[p, f] = 1 if f >= p else 0
            band = consts.tile([128, KC], bf16, tag="band")
            nc.gpsimd.memset(band, 1.0)
            nc.gpsimd.affine_select(
                out=band, in_=band, compare_op=mybir.AluOpType.is_ge, fill=0.0,
                base=0, pattern=[[1, KC]], channel_multiplier=-1,
            )

            # resident projection outputs (natural head-contiguous layout)
            qn = [[consts.tile([128, TC], bf16, tag=f"qn{tau}_{j}",
                               name=f"qn{tau}_{j}") for j in range(NJ)]
                  for tau in range(2)]
            kn = [[consts.tile([128, TC], bf16, tag=f"kn{tau}_{j}",
                               name=f"kn{tau}_{j}") for j in range(NJ)]
                  for tau in range(2)]
            vsb = [consts.tile([128, HL * 65], bf16, tag=f"v{c}", name=f"v{c}")
                   for c in range(NC_)]
            for c in range(NC_):
                vv = vsb[c].rearrange("p (h d) -> p h d", h=HL)
                nc.gpsimd.memset(vv[:, :, 64:65], 1.0)

            # ---------------- phase 1: projections ----------------
            # The rotate-half permutation matmul of part p is emitted after
            # part p+1's plain chain: it depends on an ACT copy of part p's
            # psum, and the in-order PE queue would stall on it otherwise.
            rot_pend = []

            def flush_rot(keep=0):
                while len(rot_pend) > keep:
                    pr, sb, ta, tb, dstt, js = rot_pend.pop(0)
                    nc.tensor.matmul(pr, lhsT=rmat, rhs=sb,
                                     start=True, stop=True)
                    nc.vector.tensor_mul(ta, sb, cosf[:, js])
                    nc.vector.tensor_mul(tb, pr, sinf[:, js])
                    nc.vector.tensor_add(dstt, ta, tb)

            def emit_proj_qk(jj, base, tau, dst):
                """Plain projection -> bf16 copy -> permutation matmul for
                rotate_half -> RoPE combine into dst[tau][jj]."""
                js = slice(jj * TC, (jj + 1) * TC)
                cc = base + 128 * tau
                p2 = stp.tile([128, 2 * TC], f32, tag="st",
                              name=f"pp{base}_{tau}_{jj}")
                ps = p2[:, 0:TC]
                pr = p2[:, TC:2 * TC]
                for kk in range(NKK):
                    nc.tensor.matmul(
                        ps, lhsT=w[kk][:, cc:cc + 128], rhs=xt[kk][:, js],
                        start=(kk == 0), stop=(kk == NKK - 1))
                sb = sbp.tile([128, TC], bf16, tag="sb", name=f"sb{base}_{tau}_{jj}")
                nc.scalar.copy(sb, ps)
                ta = tmp_sb.tile([128, TC], bf16, tag="ropeA",
                                 name=f"ra{base}_{tau}_{jj}")
                tb = tmp_sb.tile([128, TC], bf16, tag="ropeB",
                                 name=f"rb{base}_{tau}_{jj}")
                rot_pend.append((pr, sb, ta, tb, dst[tau][jj], js))
                flush_rot(keep=1)

            def emit_proj_v(jj):
                js0 = jj * (TC // KC)
                ps = stp.tile([128, 2 * TC], f32, tag="st", name=f"pv{jj}")
                for tt in range(TC // KC):
                    c = js0 + tt
                    for kk in range(NKK):
                        nc.tensor.matmul(
                            ps[:, tt * DL:(tt + 1) * DL],
                            lhsT=xt[kk][:, c * KC:(c + 1) * KC],
                            rhs=w[kk][:, 2 * DL:3 * DL],
                            start=(kk == 0), stop=(kk == NKK - 1))
                for tt in range(TC // KC):
                    c = js0 + tt
                    vv = vsb[c].rearrange("p (h d) -> p h d", h=HL)
                    nc.vector.tensor_copy(
                        vv[:, :, 0:64],
                        ps[:, tt * DL:(tt + 1) * DL].rearrange(
                            "p (h d) -> p h d", h=HL))

            # phase 1 proper: v of window 0, then q/k projections; the other
            # v projections ride inside the attention chunk loops where the
            # PE has slack under the ACT-bound exp cadence
            emit_proj_v(0)
            for jj in range(NJ):
                for tau in range(2):
                    emit_proj_qk(jj, 0, tau, qn)
                for tau in range(2):
                    emit_proj_qk(jj, DL, tau, kn)
            flush_rot()

            # ---------------- phase 2: attention ----------------
            def emit_y(jj, tt, cast_eng=None):
                """Output projection for q rows [jj*TC + tt*128, +128): full
                E columns, one contiguous DMA."""
                at = ats[jj]
                yp = stp.tile([128, 2 * TC], f32, tag="st", name=f"yp{jj}_{tt}")
                for n in range(2):
                    for tau in range(2):
                        nc.tensor.matmul(
                            yp[:, n * TC:(n + 1) * TC],
                            lhsT=at[tau][:, tt * KC:(tt + 1) * KC],
                            rhs=wo[tau][:, n * TC:(n + 1) * TC],
                            start=(tau == 0), stop=(tau == 1))
                ys = ysb_p.tile([128, 2 * TC], bf16, tag="y", name=f"ys{jj}_{tt}")
                if cast_eng is nc.scalar:
                    nc.scalar.copy(ys, yp)
                else:
                    nc.vector.tensor_copy(ys, yp)
                nc.sync.dma_start(
                    out=y_d[jj * TC + tt * KC:jj * TC + (tt + 1) * KC, :],
                    in_=ys)

            ats = []
            for j in range(NJ):
                nch = 4 * (j + 1)          # causal k-chunks for this window
                # previous window's 4 output-projection chunks go mid-window
                # (late enough that at[j-1] is certainly done: the in-order
                # PE queue would otherwise stall behind a premature y matmul)
                y_at = {}
                if j > 0:
                    for tt in range(4):
                        y_at.setdefault(
                            min(nch - 1, 5 + tt * max(1, (nch - 5) // 4)),
                            []).append(tt)

                at = [attn_sb.tile([128, TC], bf16, tag=f"attn{tau}",
                                   name=f"attn{tau}_{j}") for tau in range(2)]
                ats.append(at)
                av4 = avp.tile([128, 4 * TC], f32, tag="av", name=f"av_{j}")

                def emit_av(c):
                    coff = KC * (c - 4 * j) if c > 4 * j else 0
                    for h in range(HL):
                        nc.tensor.matmul(
                            av4[0:65, h * TC + coff:(h + 1) * TC],
                            lhsT=vsb[c][:, 65 * h:65 * h + 65],
                            rhs=av_est[c][h // 2][:, (h % 2) * TC + coff:
                                                  (h % 2 + 1) * TC],
                            start=(c == 0), stop=(c == nch - 1))

                av_est = {}
                for c in range(nch):
                    d = c - 4 * j          # 0..3 on the diagonal
                    coff = KC * d if d > 0 else 0
                    sts = [stp.tile([128, 2 * TC], f32, tag="st",
                                    name=f"st{j}_{c}_{i}") for i in range(2)]
                    # one K=64 matmul per head; the two heads of a tile sit
                    # on disjoint row-groups of the PE array
                    for ll in range(2):
                        for tau in range(2):
                            h = 2 * tau + ll
                            stt = sts[h // 2]
                            w0_ = (h % 2) * TC
                            nc.tensor.matmul(
                                stt[:, w0_ + coff:w0_ + TC],
                                lhsT=kn[tau][c // 4][
                                    64 * ll:64 * ll + 64,
                                    (c % 4) * KC:(c % 4 + 1) * KC],
                                rhs=qn[tau][j][64 * ll:64 * ll + 64, coff:],
                                start=True, stop=True,
                                tile_position=(64 * ll, 0))
                    ests = []
                    for i in range(2):
                        est = est_sb.tile([128, 2 * TC], bf16, tag="est",
                                          name=f"est{j}_{c}_{i}")
                        if coff:
                            nc.scalar.activation(
                                est.rearrange("p (w c) -> p w c", w=2)[
                                    :, :, coff:],
                                sts[i].rearrange("p (w c) -> p w c", w=2)[
                                    :, :, coff:],
                                EXP, scale=SCALE)
                        else:
                            nc.scalar.activation(est, sts[i], EXP, scale=SCALE)
                        ests.append(est)
                    if d >= 0:
                        for h in range(HL):
                            bs = slice((h % 2) * TC + KC * d,
                                       (h % 2) * TC + KC * (d + 1))
                            nc.vector.tensor_mul(
                                ests[h // 2][:, bs], ests[h // 2][:, bs], band)
                    av_est[c] = ests
                    # AV lagged two chunks behind the scores: keeps the PE
                    # queue fed with score matmuls while the window-boundary
                    # normalize frees the av4 accumulator (early chunks have
                    # no band-mul, so est readiness is ACT-paced only)
                    if c >= 2:
                        emit_av(c - 2)
                    if c == 2 and j + 1 < NJ:
                        emit_proj_v(j + 1)
                    for tt in y_at.get(c, ()):
                        emit_y(j - 1, tt)
                emit_av(nch - 2)
                emit_av(nch - 1)

                # softmax normalize: copy the raw sums out of psum (frees
                # av4 for the next window), then scale by the reciprocal of
                # the denominator row.  Odd heads go through a base-0 temp
                # tile: SBUF*SBUF tensor ops need equal input base
                # partitions (output partition offset is fine).
                last = j == NJ - 1

                def stage_raw():
                    tmps = {}
                    for h in range(HL):
                        hw = slice(h * TC, (h + 1) * TC)
                        if h % 2 == 0:
                            nc.vector.tensor_copy(at[h // 2][0:64, :],
                                                  av4[0:64, hw])
                        else:
                            tmp = small_sb.tile([64, TC], bf16,
                                                tag=f"atmp{h}",
                                                name=f"atmp{j}_{h}")
                            tmps[h] = tmp
                            nc.vector.tensor_copy(tmp, av4[0:64, hw])
                    return tmps

                def stage_recip():
                    rbs = []
                    for h in range(HL):
                        hw = slice(h * TC, (h + 1) * TC)
                        avd = small_sb.tile([1, TC], f32, tag="denom",
                                            name=f"avd{j}_{h}")
                        nc.vector.tensor_copy(avd, av4[64:65, hw])
                        rc = small_sb.tile([1, TC], f32, tag="recip",
                                           name=f"rc{j}_{h}")
                        nc.vector.reciprocal_approx_fast(out=rc, in_=avd)
                        rb = small_sb.tile([64, TC], f32, tag=f"rbcast{h}",
                                           name=f"rb{j}_{h}")
                        nc.gpsimd.partition_broadcast(rb, rc)
                        rbs.append(rb)
                    return rbs

                if not last:
                    # free av4 as early as possible; at-readiness has slack
                    # (y parts sit mid-next-window)
                    tmps = stage_raw()
                    rbs = stage_recip()
                    for h in range(HL):
                        dst = at[h // 2][64 * (h % 2):64 * (h % 2) + 64, :]
                        src = dst if h % 2 == 0 else tmps[h]
                        nc.vector.tensor_mul(dst, src, rbs[h])
                else:
                    # tail: at-readiness is the critical path; everything is
                    # column-split so the first output projection fires as
                    # early as possible, straight out of psum
                    for tt in range(4):
                        ks = slice(tt * KC, (tt + 1) * KC)
                        rbs = []
                        for h in range(HL):
                            co = h * TC + tt * KC
                            avd = small_sb.tile([1, KC], f32, tag="denom",
                                                name=f"avd{j}_{h}_{tt}")
                            nc.vector.tensor_copy(avd, av4[64:65, co:co + KC])
                            rc = small_sb.tile([1, KC], f32, tag="recip",
                                               name=f"rc{j}_{h}_{tt}")
                            nc.vector.reciprocal_approx_fast(out=rc, in_=avd)
                            rb = small_sb.tile([64, KC], f32,
                                               tag=f"rbcast{h}",
                                               name=f"rb{j}_{h}_{tt}")
                            nc.gpsimd.partition_broadcast(rb, rc)
                            rbs.append(rb)
                        for h in range(HL):
                            co = h * TC + tt * KC
                            nc.vector.tensor_mul(
                                at[h // 2][64 * (h % 2):64 * (h % 2) + 64,
                                           ks],
                                av4[0:64, co:co + KC], rbs[h])
                        emit_y(j, tt, cast_eng=nc.scalar)

    nc.compile()
    return nc


def _host_inputs(x, cos, sin, w_qkv, w_out):
    """Shard + lay out the full inputs for the 8 cores."""
    # natural-layout tables: row 64*l + d = cos/sin[t, d]
    cosf = np.ascontiguousarray(np.tile(cos.T, (2, 1))).astype(BF16)
    sinf = np.ascontiguousarray(np.tile(sin.T, (2, 1))).astype(BF16)

    xts = [np.ascontiguousarray(x[b].T).astype(BF16) for b in range(B)]

    # lhsT for the rotate_half permutation matmul: rot = rmat.T @ q with
    # rot[d] = -q[d+32] (d<32), q[d-32] (d>=32) per 64-row head block
    r64 = np.zeros((64, 64), dtype=np.float32)
    r64[np.arange(32) + 32, np.arange(32)] = -1.0
    r64[np.arange(32), np.arange(32) + 32] = 1.0
    rmat = np.zeros((128, 128), dtype=np.float32)
    rmat[0:64, 0:64] = r64
    rmat[64:128, 64:128] = r64
    rmat = rmat.astype(BF16)

    in_maps = []
    for core in range(8):
        b, g = divmod(core, G)
        qblk = w_qkv[:, G * g * HD:(G * g + HL) * HD]
        kblk = w_qkv[:, E + G * g * HD:E + (G * g + HL) * HD]
        vblk = w_qkv[:, 2 * E + DL * g:2 * E + DL * (g + 1)]
        wl = np.concatenate([qblk, kblk, vblk], axis=1).astype(BF16)  # (E, 768)
        wol = np.ascontiguousarray(w_out[DL * g:DL * (g + 1), :]).astype(BF16)
        in_maps.append({
            "xt": xts[b], "w": wl, "wo": wol, "cosf": cosf, "sinf": sinf,
            "rmat": rmat,
        })
    return in_maps


def kernel(x, cos, sin, w_qkv, w_out):
    from concourse import bass_utils

    if "nc" not in _CACHE:
        _CACHE["nc"] = _build_bass()
    nc = _CACHE["nc"]

    in_maps = _host_inputs(
        np.asarray(x, dtype=np.float32), np.asarray(cos, dtype=np.float32),
        np.asarray(sin, dtype=np.float32), np.asarray(w_qkv, dtype=np.float32),
        np.asarray(w_out, dtype=np.float32))

    res = bass_utils.run_bass_kernel_spmd(nc, in_maps, core_ids=list(range(8)))

    y = np.zeros((B, T, E), dtype=np.float32)
    for core in range(8):
        b = core // G
        y[b] += res.results[core]["y"].astype(np.float32)
    return y
